# revision 31
# baseline (speedup 1.0000x reference)
"""Trainium2 Bass kernel for nn_ConcatSquashLinearSA.

Math (per sample b, S=1):
    gate = sigmoid(ctx @ Wg.T + bg)          [256]
    bias = ctx @ Wb.T                        [256]
    kv   = ctx @ Wkv.T                       [256]
    E    = outer(kv, kv)                     [256,256]
    A    = softmax_rows(E)
    att  = A / (1e-9 + colsum(A))
    out  = (x @ Wl.T + bl) @ (I + att) * gate + bias

Folded for the device (U = Wl.T, g = gate, cs = colsum(A)):
    Q     = (U @ A) * (1/cs broadcast) + U   [256,256]  (tiny, on-device)
    b_fin = g * (bl + (bl @ A)/cs) + bias
    out^T = diag(g) (Q^T x^T) + b_fin
i.e. the big op is a single matmul against Q with a PER-PARTITION scale
(g[e]) and bias (b_fin[e]) applied while draining PSUM -> fp16 SBUF.

Sharding: data-parallel over batch, 2 samples per core across 8 cores.

Dataflow per core (memory-bound problem -> minimize HBM bytes):
  * x is downcast to fp8-e4m3 and pre-transposed on the HOST:
    xT [256, 32768] fp8, contraction dim k on partitions.  The PE
    consumes the fp8 rhs DIRECTLY against fp16 weights (mixed-dtype
    matmul, HW-verified exact, full 1 col/cycle rate) -- no cast DMA,
    no on-chip transposes.  8.4 MB in + 16.8 MB out per core.
  * Weight-stationary main loop: lhsT = Q k-half/e-half [128,128] fp16,
    rhs = xT chunk [128,512] fp8; psum tile [128e, 512n].  Scale+bias
    drain splits between Scalar (Identity w/ scale+bias APs) and Vector
    engines, writing fp16.  Steady state is PE-bound at ~216 ns per
    512-col matmul (warm HAM clock), ~6.9 us per 4096-col macro.
  * DVE reciprocal cost scales with free-size (~7 ns/elem/partition), so
    1/cs is computed on the TRANSPOSED [128,2] colsums (~0.16 us), then
    PE-transposed back to two [1,128] rows and PE-broadcast to [128,256]
    -- never a [*,256]-wide reciprocal (those cost ~1.76 us each).
  * Both samples' softmax chains are issued before macro 0 so their
    small PE ops fill the dependency-stall gaps of each other's chains;
    s1's Q-build goes right after macro 0, ~15 us before it is needed.
  * Loads on Sync HWDGE (512 KB fp8 tiles), steady stores on GpSimd
    SWDGE (1 MB), last-macro h0 whole on GpSimd / h1 in 1024-col pieces
    on Sync as the drains land (subtile deps) to keep the tail short.
"""

import numpy as np

B, N, DIN, DOUT, DCTX = 16, 16384, 256, 256, 131
NCORES = 8
SPC = B // NCORES           # samples per core
ROWS = SPC * N              # x rows per core
MACRO = 4096                # n-columns of xT per macro-tile
KC = DCTX + 1               # ctx rows incl. the constant-1 row (b_gate)
USE_F32R = False            # kept for test.py compat


def build_nc(rows=ROWS, use_f32r=USE_F32R):
    import concourse.bass as bass  # noqa: F401
    import concourse.tile as tile
    from concourse import bacc, mybir
    from contextlib import ExitStack

    f32 = mybir.dt.float32
    f16 = mybir.dt.float16
    f8 = mybir.dt.float8e4
    AF = mybir.ActivationFunctionType
    AX = mybir.AxisListType
    OP = mybir.AluOpType

    n_macro = rows // MACRO
    mps = rows // SPC // MACRO   # macro-tiles per sample
    NQ = MACRO // 512            # 512-col n-chunks per macro
    KB = KC - 128                # ctx rows in the second (short) pack

    nc = bacc.Bacc()
    xT_d = nc.declare_dram_parameter("xT", [256, rows], f8, isOutput=False)
    # cpa: rows = ctx-k; cols = [ctx_s(2) | Wg.T(256) | Wb.T(256) | Wkv.T(256)]
    # (b_gate rides the constant-1 ctx row)
    cpa_d = nc.declare_dram_parameter("cpa", [128, 770], f16, isOutput=False)
    cpB_d = nc.declare_dram_parameter("cpB", [KB, 770], f16, isOutput=False)
    # pki: rows = d/c-half; cols = [pk2a(514) | pk2b(514) | I128(128)]
    # pk2x = [W_layer half (256) | W_layer.T half (256) | bl2 (2)]
    pki_d = nc.declare_dram_parameter("pki", [128, 1156], f16, isOutput=False)
    outT_d = nc.declare_dram_parameter("outT", [256, rows], f16, isOutput=True)

    with tile.TileContext(nc) as tc, ExitStack() as ctx:
        consts = ctx.enter_context(tc.tile_pool(name="consts", bufs=1))
        spool = ctx.enter_context(tc.tile_pool(name="scratch", bufs=2))
        perm = ctx.enter_context(tc.tile_pool(name="persample", bufs=1))
        pps = ctx.enter_context(tc.tile_pool(name="pps", bufs=4, space="PSUM"))
        pout = ctx.enter_context(tc.tile_pool(name="pout", bufs=4, space="PSUM"))
        xin = ctx.enter_context(tc.tile_pool(name="xin", bufs=8))
        osb = ctx.enter_context(tc.tile_pool(name="osb", bufs=3))

        # ---- constants-by-memset + PE warmup (no DMA dependencies) ----
        onesrF = consts.tile([1, 128], f32, name="onesrF", tag="onesrF")
        nc.gpsimd.memset(onesrF, 1.0)
        onesc = consts.tile([128, 1], f16, name="onesc", tag="onesc")
        nc.gpsimd.memset(onesc, 1.0)
        wm = consts.tile([128, 512], f16, name="wm", tag="wm")
        nc.gpsimd.memset(wm, 0.0)
        for w in range(2):
            wp_ = pout.tile([128, 512], f32, name=f"warm{w}", tag="op")
            nc.tensor.matmul(wp_, lhsT=wm[:, 0:128], rhs=wm, start=True,
                             stop=True)

        cpA = consts.tile([128, 770], f16, name="cpA", tag="cpA")
        nc.sync.dma_start(cpA, cpa_d[:, :])
        cpB = consts.tile([KB, 770], f16, name="cpB", tag="cpB")
        nc.sync.dma_start(cpB, cpB_d[:, :])
        pki = consts.tile([128, 1156], f16, name="pki", tag="pki")
        nc.sync.dma_start(pki, pki_d[:, :])
        pk2a = pki[:, 0:514]
        pk2b = pki[:, 514:1028]
        ident = pki[:, 1028:1156]
        bl2 = pk2a[:, 512:514]

        gateT, weff, bfT = {}, {}, {}
        _sv = {}   # cross-stage per-sample intermediates

        def setup_st1(s):
            """ctx projections: kv row (for E) + gate/bias columns, and
            the gate subchain (depends only on gp, so it runs early)."""
            ckv = pps.tile([1, 256], f32, name=f"ckv{s}", tag="ps")
            nc.tensor.matmul(ckv, lhsT=cpA[:, s:s + 1], rhs=cpA[:, 514:770],
                             start=True, stop=False)
            nc.tensor.matmul(ckv, lhsT=cpB[:, s:s + 1], rhs=cpB[:, 514:770],
                             start=False, stop=True)
            kv = spool.tile([1, 256], f16, name=f"kv{s}", tag="kv")
            nc.scalar.activation(kv, ckv, AF.Identity)   # off DVE
            # gate/bias column projections fill the PE while kv lands
            gp = pps.tile([128, 2], f32, name=f"gp{s}", tag="ps")
            bt = pps.tile([128, 2], f32, name=f"bt{s}", tag="ps")
            for h in range(2):
                c0 = 2 + 128 * h
                nc.tensor.matmul(gp[:, h:h + 1], lhsT=cpA[:, c0:c0 + 128],
                                 rhs=cpA[:, s:s + 1], start=True, stop=False)
                nc.tensor.matmul(gp[:, h:h + 1], lhsT=cpB[:, c0:c0 + 128],
                                 rhs=cpB[:, s:s + 1], start=False, stop=True)
                nc.tensor.matmul(bt[:, h:h + 1],
                                 lhsT=cpA[:, c0 + 256:c0 + 384],
                                 rhs=cpA[:, s:s + 1], start=True, stop=False)
                nc.tensor.matmul(bt[:, h:h + 1],
                                 lhsT=cpB[:, c0 + 256:c0 + 384],
                                 rhs=cpB[:, s:s + 1], start=False, stop=True)
            # gate = 1/(1+e^-x); runs on ACT/GPSIMD/DVE while PE continues
            eg = spool.tile([128, 2], f32, name=f"eg{s}", tag="eg")
            nc.scalar.activation(eg, gp, AF.Exp, scale=-1.0)
            ga = spool.tile([128, 2], f32, name=f"ga{s}", tag="ga")
            nc.gpsimd.tensor_scalar_add(ga, eg, 1.0)
            gateT[s] = perm.tile([128, 2], f32, name=f"gateT{s}",
                                 tag=f"gateT{s}")
            nc.vector.reciprocal(gateT[s], ga)
            btS = spool.tile([128, 2], f32, name=f"btS{s}", tag="btS")
            nc.scalar.activation(btS, bt, AF.Identity)   # off DVE (ACT
            # reads PSUM; gpsimd cannot)
            _sv[s] = {"kv": kv, "btS": btS}

        def setup_st2(s):
            """E = outer(kv, kv); row softmax (|E| small: no max-sub)."""
            kv = _sv[s]["kv"]
            A = {}
            for i in range(2):
                E = pps.tile([128, 256], f32, name=f"E{s}{i}", tag="ps")
                nc.tensor.matmul(E, lhsT=kv[0:1, 128 * i:128 * (i + 1)],
                                 rhs=kv, start=True, stop=True)
                expE = spool.tile([128, 256], f32, name=f"expE{s}{i}",
                                  tag="expE")
                rs = spool.tile([128, 1], f32, name=f"rs{s}{i}", tag="rs")
                # exp emits its row-sum via the ACT accumulator: no DVE
                # reduce hop on the critical path
                nc.scalar.activation(expE, E, AF.Exp, accum_out=rs)
                rc = spool.tile([128, 1], f32, name=f"rc{s}{i}", tag="rc")
                nc.vector.reciprocal(rc, rs)
                A[i] = spool.tile([128, 256], f16, name=f"A{s}{i}",
                                  tag=f"A{s}{i}")
                nc.vector.tensor_scalar_mul(A[i], expE, rc)
            _sv[s]["A"] = A

        def setup_st3(s):
            """colsum row -> ~5x-fast approx reciprocal -> single fp32
            PE broadcast; Q = (U@A) * (1/cs) + U (fp16)."""
            A = _sv[s]["A"]
            csr = pps.tile([1, 256], f32, name=f"csr{s}", tag="ps")
            nc.tensor.matmul(csr, lhsT=onesc, rhs=A[0], start=True, stop=False)
            nc.tensor.matmul(csr, lhsT=onesc, rhs=A[1], start=False, stop=True)
            rr = spool.tile([1, 256], f32, name=f"rr{s}", tag="rr")
            nc.vector.reciprocal_approx_fast(rr, csr)   # 18-bit, ~5x faster
            # wp = U @ A runs on PE while rr resolves
            wpj = {}
            for j in range(2):
                wpj[j] = pps.tile([128, 256], f32, name=f"wp{s}{j}", tag="ps")
                nc.tensor.matmul(wpj[j], lhsT=pk2a[:, 128 * j:128 * (j + 1)],
                                 rhs=A[0], start=True, stop=False)
                nc.tensor.matmul(wpj[j], lhsT=pk2b[:, 128 * j:128 * (j + 1)],
                                 rhs=A[1], start=False, stop=True)
            cb = pps.tile([128, 256], f32, name=f"cb{s}", tag="ps")
            nc.tensor.matmul(cb, lhsT=onesrF, rhs=rr, start=True, stop=True)
            CSi = spool.tile([128, 256], f32, name=f"CSi{s}", tag="CSi")
            nc.scalar.activation(CSi, cb, AF.Identity)  # DVE allows only
            # one PSUM operand per op, so land 1/cs in SBUF first
            for j in range(2):   # Q = wp * (1/cs) + U  (fp16)
                qm = spool.tile([128, 256], f16, name=f"qm{s}{j}", tag="qm")
                nc.vector.tensor_mul(qm, wpj[j], CSi)
                weff[(s, j)] = perm.tile([128, 256], f16, name=f"weff{s}{j}",
                                         tag=f"weff{s}{j}")
                U_half = pk2a[:, 256:512] if j == 0 else pk2b[:, 256:512]
                nc.gpsimd.tensor_add(weff[(s, j)], qm, U_half)

        def setup_st4(s):
            """b_fin columns: g*(bl + (bl@A)/cs) + bias."""
            A, btS = _sv[s]["A"], _sv[s]["btS"]
            # transposed colsums ct[p,h] = cs[128h+p]: the cheap [128,2]
            # reciprocal layout
            ct = pps.tile([128, 2], f32, name=f"ct{s}", tag="ps")
            qa = pps.tile([128, 2], f32, name=f"qa{s}", tag="ps")
            for h in range(2):
                hs = slice(128 * h, 128 * (h + 1))
                nc.tensor.matmul(ct[:, h:h + 1], lhsT=A[0][:, hs],
                                 rhs=onesc, start=True, stop=False)
                nc.tensor.matmul(ct[:, h:h + 1], lhsT=A[1][:, hs],
                                 rhs=onesc, start=False, stop=True)
                nc.tensor.matmul(qa[:, h:h + 1], lhsT=A[0][:, hs],
                                 rhs=bl2[:, 0:1], start=True, stop=False)
                nc.tensor.matmul(qa[:, h:h + 1], lhsT=A[1][:, hs],
                                 rhs=bl2[:, 1:2], start=False, stop=True)
            rcT = spool.tile([128, 2], f32, name=f"rcT{s}", tag="rcT")
            nc.vector.reciprocal(rcT, ct)
            f1 = spool.tile([128, 2], f32, name=f"f1{s}", tag="f1")
            nc.vector.tensor_mul(f1, qa, rcT)   # qa is PSUM: DVE, not gpsimd
            f2 = spool.tile([128, 2], f32, name=f"f2{s}", tag="f2")
            nc.gpsimd.tensor_add(f2, f1, bl2)
            f3 = spool.tile([128, 2], f32, name=f"f3{s}", tag="f3")
            nc.gpsimd.tensor_mul(f3, f2, gateT[s])
            bfT[s] = perm.tile([128, 2], f32, name=f"bfT{s}", tag=f"bfT{s}")
            nc.gpsimd.tensor_add(bfT[s], f3, btS)

        def load(n0, cols):
            """Issue a macro's x loads; queued Sync-FIFO ahead of any tail
            store so a store's drain-wait can't head-of-line block them."""
            xa = xin.tile([128, cols], f8, name="xa", tag="xa")
            nc.sync.dma_start(xa, xT_d[0:128, n0:n0 + cols])
            xb = xin.tile([128, cols], f8, name="xb", tag="xb")
            nc.sync.dma_start(xb, xT_d[128:256, n0:n0 + cols])
            return xa, xb

        def macro(n0, cols, tail, xab):
            """One macro tile of the streaming matmul ([128, cols] x 2)."""
            s = (n0 * SPC) // rows
            xa, xb = xab
            for h in range(2):
                gcol = gateT[s][:, h:h + 1]
                bcol = bfT[s][:, h:h + 1]
                ot = osb.tile([128, cols], f16, name=f"ot{h}", tag=f"ot{h}")
                for q in range(cols // 512):
                    op = pout.tile([128, 512], f32, name="op", tag="op")
                    nc.tensor.matmul(op, lhsT=weff[(s, 0)][:, 128 * h:128 * (h + 1)],
                                     rhs=xa[:, 512 * q:512 * (q + 1)],
                                     start=True, stop=False)
                    nc.tensor.matmul(op, lhsT=weff[(s, 1)][:, 128 * h:128 * (h + 1)],
                                     rhs=xb[:, 512 * q:512 * (q + 1)],
                                     start=False, stop=True)
                    dst = ot[:, 512 * q:512 * (q + 1)]
                    if q % 2 == 0:
                        nc.scalar.activation(dst, op, AF.Identity,
                                             bias=bcol, scale=gcol)
                    else:
                        nc.vector.tensor_scalar(dst, op, gcol, bcol,
                                                op0=OP.mult, op1=OP.add)
                dram = outT_d[128 * h:128 * (h + 1), n0:n0 + cols]
                if tail:
                    # tail minis go on the HWDGE queues (Scalar for h0,
                    # Sync for h1): both are idle by then and drain fast
                    # at teardown, while gpsimd's SWDGE queue (its last
                    # 1 MB store ~6 us earlier) empties in the shadow
                    (nc.scalar if h == 0 else nc.sync).dma_start(dram, ot)
                else:
                    # steady-state stores on the Pool SWDGE queue, in
                    # parallel with Sync's load issue
                    nc.gpsimd.dma_start(dram, ot)

        # The last 4096-col macro is tapered into 2048+1024+1024 minis so
        # the final stores are small and in flight before compute ends
        # (a trailing 2 MB store costs ~8 us of post-compute drain).
        segs, n0 = [], 0
        for cols in [MACRO] * (n_macro - 1) + [2048, 1024, 1024]:
            segs.append((n0, cols))
            n0 += cols
        assert n0 == rows

        # Both samples' softmax chains issue before macro 0 (their small
        # PE ops fill each other's dependency-stall gaps); s1's Q-build
        # right after macro 0, ~15 us before its first use in macro 4.
        setup_st1(0)
        setup_st2(0)
        setup_st1(1)
        setup_st2(1)
        setup_st3(0)
        setup_st4(0)
        setup_st3(1)
        setup_st4(1)
        xabs = {n0: load(n0, cols) for n0, cols in segs}
        for n0, cols in segs:
            macro(n0, cols, cols < MACRO, xabs[n0])

    nc.finalize()
    return nc


def prep_host_inputs(ctx, x, W_layer, b_layer, W_bias, W_gate, b_gate, W_kv,
                     rows=ROWS):
    """Build the per-core in_maps (host-side sharding + fp8/fp16 relayout)."""
    import ml_dtypes

    ctx = np.asarray(ctx, np.float32)
    W_layer = np.asarray(W_layer, np.float32)
    b_layer = np.asarray(b_layer, np.float32)
    W_bias = np.asarray(W_bias, np.float32)
    W_gate = np.asarray(W_gate, np.float32)
    b_gate = np.asarray(b_gate, np.float32)
    W_kv = np.asarray(W_kv, np.float32)
    x8 = np.asarray(x, np.float32).astype(ml_dtypes.float8_e4m3)

    wcatT = np.zeros((KC, 768), np.float32)
    wcatT[:DCTX, 0:256] = W_gate.T
    wcatT[:DCTX, 256:512] = W_bias.T
    wcatT[:DCTX, 512:768] = W_kv.T
    wcatT[DCTX, 0:256] = b_gate        # paired with the constant-1 ctx row

    bl2 = b_layer.reshape(2, 128).T            # bl2[p, h] = bl[128h+p]
    pk2a = np.concatenate([W_layer[0:128, :], W_layer.T[0:128, :], bl2],
                          axis=1)
    pk2b = np.concatenate([W_layer[128:256, :], W_layer.T[128:256, :], bl2],
                          axis=1)
    pki = np.concatenate([pk2a, pk2b, np.eye(128, dtype=np.float32)],
                         axis=1).astype(np.float16)   # [128, 1156]
    in_maps = []
    for c in range(NCORES):
        ctxc = np.zeros((KC, SPC), np.float32)
        for k in range(SPC):
            ctxc[:DCTX, k] = ctx[SPC * c + k, 0]
            ctxc[DCTX, k] = 1.0
        cpack = np.concatenate([ctxc, wcatT], axis=1)   # [KC, 770]
        xT = np.ascontiguousarray(
            x8[SPC * c:SPC * (c + 1)].reshape(rows, DIN).T)
        in_maps.append({
            "xT": xT,
            "cpa": np.ascontiguousarray(cpack[0:128].astype(np.float16)),
            "cpB": np.ascontiguousarray(cpack[128:KC].astype(np.float16)),
            "pki": np.ascontiguousarray(pki),
        })
    return in_maps


def kernel(ctx, x, W_layer, b_layer, W_bias, W_gate, b_gate, W_kv):
    from concourse.bass_utils import run_bass_kernel_spmd

    nc = build_nc(ROWS)
    in_maps = prep_host_inputs(ctx, x, W_layer, b_layer, W_bias, W_gate,
                               b_gate, W_kv)
    res = run_bass_kernel_spmd(nc, in_maps, core_ids=list(range(NCORES)))
    out = np.empty((B, N, DOUT), np.float32)
    for c in range(NCORES):
        outT = res.results[c]["outT"]           # [256, ROWS] fp16
        out[SPC * c:SPC * (c + 1)] = (
            outT.T.reshape(SPC, N, DOUT).astype(np.float32))
    return out
